# revision 2
# baseline (speedup 1.0000x reference)
"""Trainium2 Bass kernel for nn_EntityEmbedding_18433999634983.

Reference semantics: RGCN-style basis-decomposed message passing with
scatter-mean aggregation, but the final output is only row `unseen_index`
of the aggregated node matrix:

    out = relu( (sum_{e: dst[e]==u} msg_e) / max(#{e: dst[e]==u}, 1) )
    msg_e = sum_b att[edge_type[e], b] * concat(x[src[e]], rel_emb[rel_index[e]]) @ basis[b]

Only edges with dst == unseen_index contribute (~20 of 1M for uniform dst).
The kernel therefore:
  1. shards edges across 8 cores; each core streams its dst shard through
     SBUF as a [128, 977] tile, compares against unseen_index (mask),
     reduce-sums the mask for the count, and extracts the matching edge
     indices with a masked-iota + per-partition top-8 max (all on device);
  2. gathers (src, edge_type, rel_index) rows for the matched edges with
     indirect DMA, then node_id[src], then entity_table / relation_embedding
     / att rows (two rounds of 128 candidate slots = up to 2 matches per
     partition, far above the observed max of 1);
  3. contracts the gathered rows with tiny PE matmuls:
         v_b[96] = sum_e valid_e * att[et_e, b] * x_cat_e
         partial[64] = sum_b basis[b].T @ v_b
     and counts via a ones-matmul;
  4. host sums the 8 per-core partial[64] vectors and counts, divides,
     applies ReLU.
"""

import os

import numpy as np

# ---- problem constants (hardcoded per spec) ----
N_CORES = 8
E = 1_000_000
S = E // N_CORES          # 125_000 edges per core
P = 128
F = 977                   # ceil(S / P)
PAD = P * F               # 125_056
N_NODES = 50_000
N_ENT = 200_000
D_E = 64
D_R = 32
IN_CH = D_E + D_R         # 96
N_REL2 = 400              # 2R (att rows)
N_REL = 200               # R  (relation_embedding rows)
N_BASES = 2
ROUNDS = 2                # gather rounds (max matches per partition covered)

_CACHE = {}
LAST_RESULTS = None       # BassKernelResults of the most recent run (for test.py)


def _build_program():
    import concourse.bacc as bacc
    import concourse.tile as tile
    import concourse.mybir as mybir
    from concourse.bass import IndirectOffsetOnAxis

    f32 = mybir.dt.float32
    i32 = mybir.dt.int32

    nc = bacc.Bacc("TRN2", target_bir_lowering=False, debug=False)

    dst_d = nc.dram_tensor("dst", [P, F], i32, kind="ExternalInput")
    packed_d = nc.dram_tensor("packed", [S, 4], i32, kind="ExternalInput")
    nid_d = nc.dram_tensor("node_id", [N_NODES, 1], i32, kind="ExternalInput")
    ent_d = nc.dram_tensor("entity", [N_ENT, D_E], f32, kind="ExternalInput")
    rel_d = nc.dram_tensor("rel", [N_REL, D_R], f32, kind="ExternalInput")
    att_d = nc.dram_tensor("att", [N_REL2, N_BASES], f32, kind="ExternalInput")
    bas_d = [
        nc.dram_tensor(f"basis{b}", [IN_CH, D_E], f32, kind="ExternalInput")
        for b in range(N_BASES)
    ]
    uns_d = nc.dram_tensor("unseen", [1, 1], i32, kind="ExternalInput")

    part_d = nc.dram_tensor("partial", [D_E, 1], f32, kind="ExternalOutput")
    cnt_d = nc.dram_tensor("cnt", [1, 1], f32, kind="ExternalOutput")

    X = mybir.AxisListType.X
    OP = mybir.AluOpType

    with tile.TileContext(nc) as tc:
        with (
            tc.tile_pool(name="sbuf", bufs=1) as sb,
            tc.tile_pool(name="psum", bufs=1, space="PSUM") as ps,
        ):
            # ---- dense phase: mask + count + masked-iota top-8 ----
            dst_t = sb.tile([P, F], i32)
            nc.sync.dma_start(dst_t[:], dst_d[:])

            u1 = sb.tile([1, 1], i32)
            nc.sync.dma_start(u1[:], uns_d[:])
            ub = sb.tile([P, 1], i32)
            nc.gpsimd.partition_broadcast(ub[:], u1[:])

            mask = sb.tile([P, F], f32)
            nc.vector.tensor_tensor(
                out=mask[:], in0=dst_t[:], in1=ub[:].to_broadcast([P, F]),
                op=OP.is_equal,
            )
            cntp = sb.tile([P, 1], f32)
            nc.vector.reduce_sum(out=cntp[:], in_=mask[:], axis=X)

            iota_i = sb.tile([P, F], i32)
            nc.gpsimd.iota(iota_i[:], pattern=[[1, F]], base=1, channel_multiplier=F)
            iota_f = sb.tile([P, F], f32)
            nc.vector.tensor_copy(iota_f[:], iota_i[:])
            mi = sb.tile([P, F], f32)
            nc.vector.tensor_tensor(out=mi[:], in0=mask[:], in1=iota_f[:], op=OP.mult)

            top8 = sb.tile([P, 8], f32)
            nc.vector.max(top8[:], mi[:])

            # ---- params in SBUF ----
            bas_t = []
            for b in range(N_BASES):
                t = sb.tile([IN_CH, D_E], f32, tag=f"basis{b}")
                nc.sync.dma_start(t[:], bas_d[b][:])
                bas_t.append(t)
            ones = sb.tile([P, 1], f32)
            nc.vector.memset(ones[:], 1.0)

            # ---- sparse gather rounds ----
            xcats = []
            coefs = []  # coefs[r][b]
            for r in range(ROUNDS):
                col = top8[:, r:r + 1]
                val = sb.tile([P, 1], f32, tag=f"val{r}")
                nc.vector.tensor_scalar(
                    out=val[:], in0=col, scalar1=0.5, scalar2=None, op0=OP.is_gt,
                )
                idf = sb.tile([P, 1], f32, tag=f"idf{r}")
                nc.vector.tensor_scalar(
                    out=idf[:], in0=col, scalar1=-1.0, scalar2=0.0,
                    op0=OP.add, op1=OP.max,
                )
                idx = sb.tile([P, 1], i32, tag=f"idx{r}")
                nc.vector.tensor_copy(idx[:], idf[:])

                pk = sb.tile([P, 4], i32, tag=f"pk{r}")
                nc.gpsimd.indirect_dma_start(
                    out=pk[:], out_offset=None, in_=packed_d[:],
                    in_offset=IndirectOffsetOnAxis(ap=idx[:, :1], axis=0),
                )
                nidg = sb.tile([P, 1], i32, tag=f"nidg{r}")
                nc.gpsimd.indirect_dma_start(
                    out=nidg[:], out_offset=None, in_=nid_d[:],
                    in_offset=IndirectOffsetOnAxis(ap=pk[:, 0:1], axis=0),
                )
                xcat = sb.tile([P, IN_CH], f32, tag=f"xcat{r}")
                nc.gpsimd.indirect_dma_start(
                    out=xcat[:, 0:D_E], out_offset=None, in_=ent_d[:],
                    in_offset=IndirectOffsetOnAxis(ap=nidg[:, :1], axis=0),
                )
                nc.gpsimd.indirect_dma_start(
                    out=xcat[:, D_E:IN_CH], out_offset=None, in_=rel_d[:],
                    in_offset=IndirectOffsetOnAxis(ap=pk[:, 2:3], axis=0),
                )
                attg = sb.tile([P, N_BASES], f32, tag=f"attg{r}")
                nc.gpsimd.indirect_dma_start(
                    out=attg[:], out_offset=None, in_=att_d[:],
                    in_offset=IndirectOffsetOnAxis(ap=pk[:, 1:2], axis=0),
                )
                cs = []
                for b in range(N_BASES):
                    c = sb.tile([P, 1], f32, tag=f"coef{r}{b}")
                    nc.vector.tensor_tensor(
                        out=c[:], in0=attg[:, b:b + 1], in1=val[:], op=OP.mult,
                    )
                    cs.append(c)
                xcats.append(xcat)
                coefs.append(cs)

            # ---- contraction matmuls ----
            out_ps = ps.tile([D_E, 1], f32)
            for b in range(N_BASES):
                v_ps = ps.tile([IN_CH, 1], f32, tag=f"v{b}")
                for r in range(ROUNDS):
                    nc.tensor.matmul(
                        out=v_ps[:], lhsT=xcats[r][:], rhs=coefs[r][b][:],
                        start=(r == 0), stop=(r == ROUNDS - 1),
                    )
                v_sb = sb.tile([IN_CH, 1], f32, tag=f"vs{b}")
                nc.vector.tensor_copy(v_sb[:], v_ps[:])
                nc.tensor.matmul(
                    out=out_ps[:], lhsT=bas_t[b][:], rhs=v_sb[:],
                    start=(b == 0), stop=(b == N_BASES - 1),
                )

            cnt_ps = ps.tile([1, 1], f32)
            nc.tensor.matmul(
                out=cnt_ps[:], lhsT=cntp[:], rhs=ones[:], start=True, stop=True,
            )

            po = sb.tile([D_E, 1], f32)
            nc.vector.tensor_copy(po[:], out_ps[:])
            nc.sync.dma_start(part_d[:], po[:])
            co = sb.tile([1, 1], f32)
            nc.vector.tensor_copy(co[:], cnt_ps[:])
            nc.sync.dma_start(cnt_d[:], co[:])

    nc.finalize()
    return nc


def _get_nc():
    if "nc" not in _CACHE:
        _CACHE["nc"] = _build_program()
    return _CACHE["nc"]


def kernel(**inputs) -> np.ndarray:
    global LAST_RESULTS
    from concourse import bass_utils

    ent = np.ascontiguousarray(np.asarray(inputs["entity_table"], np.float32))
    rel = np.ascontiguousarray(np.asarray(inputs["relation_embedding"], np.float32))
    att = np.ascontiguousarray(np.asarray(inputs["att"], np.float32))
    basis = np.asarray(inputs["basis"], np.float32)
    node_id = np.asarray(inputs["node_id"]).astype(np.int32).reshape(N_NODES, 1)
    edge_index = np.asarray(inputs["edge_index"]).astype(np.int32)
    edge_type = np.asarray(inputs["edge_type"]).astype(np.int32)
    rel_index = np.asarray(inputs["rel_index"]).astype(np.int32)
    unseen = np.asarray(inputs["unseen_index"]).astype(np.int32).reshape(1, 1)

    src, dst = edge_index[0], edge_index[1]
    b0 = np.ascontiguousarray(basis[0])
    b1 = np.ascontiguousarray(basis[1])

    in_maps = []
    for c in range(N_CORES):
        sl = slice(c * S, (c + 1) * S)
        dpad = np.full((PAD,), -1, np.int32)
        dpad[:S] = dst[sl]
        packed = np.zeros((S, 4), np.int32)
        packed[:, 0] = src[sl]
        packed[:, 1] = edge_type[sl]
        packed[:, 2] = rel_index[sl]
        in_maps.append({
            "dst": dpad.reshape(P, F),
            "packed": packed,
            "node_id": node_id,
            "entity": ent,
            "rel": rel,
            "att": att,
            "basis0": b0,
            "basis1": b1,
            "unseen": unseen,
        })

    res = bass_utils.run_bass_kernel_spmd(
        _get_nc(), in_maps, core_ids=list(range(N_CORES)),
    )
    LAST_RESULTS = res

    total = np.zeros(D_E, np.float32)
    cnt = 0.0
    for r in res.results:
        total = total + r["partial"][:, 0]
        cnt += float(r["cnt"][0, 0])
    out = np.maximum(total / np.float32(max(cnt, 1.0)), np.float32(0.0))
    return out.astype(np.float32)


# revision 3
# speedup vs baseline: 1.0843x; 1.0843x over previous
"""Trainium2 Bass kernel for nn_EntityEmbedding_18433999634983.

Reference semantics: RGCN-style basis-decomposed message passing with
scatter-mean aggregation, but the final output is only row `unseen_index`
of the aggregated node matrix:

    out = relu( (sum_{e: dst[e]==u} msg_e) / max(#{e: dst[e]==u}, 1) )
    msg_e = sum_b att[edge_type[e], b] * concat(x[src[e]], rel_emb[rel_index[e]]) @ basis[b]

Only edges with dst == unseen_index contribute (~20 of 1M for uniform dst).
Per core (edges sharded 8 ways):
  1. stream the dst shard through SBUF as a [128, 977] tile, compare against
     unseen_index (mask), multiply by an iota constant, and extract the
     matching edge indices with a per-partition top-8 max — all on device;
  2. gather (src, edge_type) rows for the matched edges with indirect DMA,
     then node_id[src], then entity_table rows and combined att||rel_emb
     rows (two rounds of 128 candidate slots = up to 2 matches per
     partition, above the observed max of 1);
  3. contract the gathered rows with tiny PE matmuls:
         v_b[96] = sum_e valid_e * att[et_e, b] * x_cat_e
         partial[64] = sum_b basis[b].T @ v_b
     (split into entity/relation halves so everything stays
     partition-aligned), count matches via a ones-matmul on the top-8
     validity mask;
  4. host sums the 8 per-core partial[64] vectors and counts, divides,
     applies ReLU.

The combined att||rel table assumes rel_index[e] == edge_type[e] % 200
(true by construction of the reference generator: edge_type = cat(rel,
rel+R), rel_index = cat(rel, rel)); the host verifies this and falls back
to a 3-gather program variant if it ever fails.
"""

import numpy as np

# ---- problem constants (hardcoded per spec) ----
N_CORES = 8
E = 1_000_000
S = E // N_CORES          # 125_000 edges per core
P = 128
F = 977                   # ceil(S / P)
PAD = P * F               # 125_056
N_NODES = 50_000
N_ENT = 200_000
D_E = 64
D_R = 32
IN_CH = D_E + D_R         # 96
N_REL2 = 400              # 2R (att rows)
N_REL = 200               # R  (relation_embedding rows)
N_BASES = 2
COMB_W = 36               # att (2) + rel_emb (32) + pad (2)
ROUNDS = 2                # gather rounds (max matches per partition covered)

_CACHE = {}
LAST_RESULTS = None       # BassKernelResults of the most recent run (for test.py)


def _build_program(fused_rel: bool):
    import concourse.bacc as bacc
    import concourse.tile as tile
    import concourse.mybir as mybir
    from concourse.bass import IndirectOffsetOnAxis

    f32 = mybir.dt.float32
    i32 = mybir.dt.int32

    nc = bacc.Bacc("TRN2", target_bir_lowering=False, debug=False)

    pk_w = 2 if fused_rel else 4
    dst_d = nc.dram_tensor("dst", [P, F], i32, kind="ExternalInput")
    iota_d = nc.dram_tensor("iota", [P, F], f32, kind="ExternalInput")
    packed_d = nc.dram_tensor("packed", [S, pk_w], i32, kind="ExternalInput")
    nid_d = nc.dram_tensor("node_id", [N_NODES, 1], i32, kind="ExternalInput")
    ent_d = nc.dram_tensor("entity", [N_ENT, D_E], f32, kind="ExternalInput")
    comb_d = nc.dram_tensor("comb", [N_REL2, COMB_W], f32, kind="ExternalInput")
    rel_d = (None if fused_rel else
             nc.dram_tensor("rel", [N_REL, D_R], f32, kind="ExternalInput"))
    be_d = [nc.dram_tensor(f"basis_ent{b}", [D_E, D_E], f32, kind="ExternalInput")
            for b in range(N_BASES)]
    br_d = [nc.dram_tensor(f"basis_rel{b}", [D_R, D_E], f32, kind="ExternalInput")
            for b in range(N_BASES)]
    uns_d = nc.dram_tensor("unseen", [P, 1], i32, kind="ExternalInput")

    part_d = nc.dram_tensor("partial", [D_E, 1], f32, kind="ExternalOutput")
    cnt_d = nc.dram_tensor("cnt", [1, 1], f32, kind="ExternalOutput")

    X = mybir.AxisListType.X
    OP = mybir.AluOpType

    with tile.TileContext(nc) as tc:
        with (
            tc.tile_pool(name="sbuf", bufs=1) as sb,
            tc.tile_pool(name="psum", bufs=1, space="PSUM") as ps,
        ):
            # ---- dense phase: mask, masked-iota, top-8 extraction ----
            dst_t = sb.tile([P, F], i32)
            nc.sync.dma_start(dst_t[:], dst_d[:])
            iota_t = sb.tile([P, F], f32)
            nc.sync.dma_start(iota_t[:], iota_d[:])
            ub = sb.tile([P, 1], i32)
            nc.sync.dma_start(ub[:], uns_d[:])

            mask = sb.tile([P, F], f32)
            nc.vector.tensor_tensor(
                out=mask[:], in0=dst_t[:], in1=ub[:].to_broadcast([P, F]),
                op=OP.is_equal,
            )
            mi = sb.tile([P, F], f32)
            nc.vector.tensor_tensor(out=mi[:], in0=mask[:], in1=iota_t[:], op=OP.mult)
            top8 = sb.tile([P, 8], f32)
            nc.vector.max(top8[:], mi[:])

            # exact match count (top-8 per partition is exact for <=8/partition)
            c8 = sb.tile([P, 8], f32)
            nc.vector.tensor_scalar(
                out=c8[:], in0=top8[:], scalar1=0.5, scalar2=None, op0=OP.is_gt,
            )
            cntp = sb.tile([P, 1], f32)
            nc.vector.reduce_sum(out=cntp[:], in_=c8[:], axis=X)

            # ---- params in SBUF ----
            be_t, br_t = [], []
            for b in range(N_BASES):
                t = sb.tile([D_E, D_E], f32, tag=f"be{b}")
                nc.sync.dma_start(t[:], be_d[b][:])
                be_t.append(t)
                t = sb.tile([D_R, D_E], f32, tag=f"br{b}")
                nc.sync.dma_start(t[:], br_d[b][:])
                br_t.append(t)
            ones = sb.tile([P, 1], f32)
            nc.vector.memset(ones[:], 1.0)

            # ---- sparse gather rounds ----
            ents, rels, coefs = [], [], []
            for r in range(ROUNDS):
                col = top8[:, r:r + 1]
                val = sb.tile([P, 1], f32, tag=f"val{r}")
                nc.vector.tensor_scalar(
                    out=val[:], in0=col, scalar1=0.5, scalar2=None, op0=OP.is_gt,
                )
                idf = sb.tile([P, 1], f32, tag=f"idf{r}")
                nc.vector.tensor_scalar(
                    out=idf[:], in0=col, scalar1=-1.0, scalar2=0.0,
                    op0=OP.add, op1=OP.max,
                )
                idx = sb.tile([P, 1], i32, tag=f"idx{r}")
                nc.vector.tensor_copy(idx[:], idf[:])

                pk = sb.tile([P, pk_w], i32, tag=f"pk{r}")
                nc.gpsimd.indirect_dma_start(
                    out=pk[:], out_offset=None, in_=packed_d[:],
                    in_offset=IndirectOffsetOnAxis(ap=idx[:, :1], axis=0),
                )
                nidg = sb.tile([P, 1], i32, tag=f"nidg{r}")
                nc.gpsimd.indirect_dma_start(
                    out=nidg[:], out_offset=None, in_=nid_d[:],
                    in_offset=IndirectOffsetOnAxis(ap=pk[:, 0:1], axis=0),
                )
                entg = sb.tile([P, D_E], f32, tag=f"entg{r}")
                nc.gpsimd.indirect_dma_start(
                    out=entg[:], out_offset=None, in_=ent_d[:],
                    in_offset=IndirectOffsetOnAxis(ap=nidg[:, :1], axis=0),
                )
                combg = sb.tile([P, COMB_W], f32, tag=f"combg{r}")
                nc.gpsimd.indirect_dma_start(
                    out=combg[:], out_offset=None, in_=comb_d[:],
                    in_offset=IndirectOffsetOnAxis(ap=pk[:, 1:2], axis=0),
                )
                if fused_rel:
                    relg = combg[:, 2:2 + D_R]
                else:
                    relg_t = sb.tile([P, D_R], f32, tag=f"relg{r}")
                    nc.gpsimd.indirect_dma_start(
                        out=relg_t[:], out_offset=None, in_=rel_d[:],
                        in_offset=IndirectOffsetOnAxis(ap=pk[:, 2:3], axis=0),
                    )
                    relg = relg_t[:]
                cs = []
                for b in range(N_BASES):
                    c = sb.tile([P, 1], f32, tag=f"coef{r}{b}")
                    nc.vector.tensor_tensor(
                        out=c[:], in0=combg[:, b:b + 1], in1=val[:], op=OP.mult,
                    )
                    cs.append(c)
                ents.append(entg)
                rels.append(relg)
                coefs.append(cs)

            # ---- contraction matmuls ----
            out_ps = ps.tile([D_E, 1], f32)
            mm = 0
            for b in range(N_BASES):
                ve_ps = ps.tile([D_E, 1], f32, tag=f"ve{b}")
                vr_ps = ps.tile([D_R, 1], f32, tag=f"vr{b}")
                for r in range(ROUNDS):
                    nc.tensor.matmul(
                        out=ve_ps[:], lhsT=ents[r][:], rhs=coefs[r][b][:],
                        start=(r == 0), stop=(r == ROUNDS - 1),
                    )
                    nc.tensor.matmul(
                        out=vr_ps[:], lhsT=rels[r], rhs=coefs[r][b][:],
                        start=(r == 0), stop=(r == ROUNDS - 1),
                    )
                ve_sb = sb.tile([D_E, 1], f32, tag=f"ves{b}")
                nc.vector.tensor_copy(ve_sb[:], ve_ps[:])
                vr_sb = sb.tile([D_R, 1], f32, tag=f"vrs{b}")
                nc.vector.tensor_copy(vr_sb[:], vr_ps[:])
                nc.tensor.matmul(
                    out=out_ps[:], lhsT=be_t[b][:], rhs=ve_sb[:],
                    start=(mm == 0), stop=False,
                )
                mm += 1
                nc.tensor.matmul(
                    out=out_ps[:], lhsT=br_t[b][:], rhs=vr_sb[:],
                    start=False, stop=(b == N_BASES - 1),
                )
                mm += 1

            cnt_ps = ps.tile([1, 1], f32)
            nc.tensor.matmul(
                out=cnt_ps[:], lhsT=cntp[:], rhs=ones[:], start=True, stop=True,
            )

            po = sb.tile([D_E, 1], f32)
            nc.vector.tensor_copy(po[:], out_ps[:])
            nc.sync.dma_start(part_d[:], po[:])
            co = sb.tile([1, 1], f32)
            nc.vector.tensor_copy(co[:], cnt_ps[:])
            nc.sync.dma_start(cnt_d[:], co[:])

    nc.finalize()
    return nc


def _get_nc(fused_rel: bool):
    key = ("nc", fused_rel)
    if key not in _CACHE:
        _CACHE[key] = _build_program(fused_rel)
    return _CACHE[key]


def kernel(**inputs) -> np.ndarray:
    global LAST_RESULTS
    from concourse import bass_utils

    ent = np.ascontiguousarray(np.asarray(inputs["entity_table"], np.float32))
    rel = np.ascontiguousarray(np.asarray(inputs["relation_embedding"], np.float32))
    att = np.ascontiguousarray(np.asarray(inputs["att"], np.float32))
    basis = np.asarray(inputs["basis"], np.float32)
    node_id = np.asarray(inputs["node_id"]).astype(np.int32).reshape(N_NODES, 1)
    edge_index = np.asarray(inputs["edge_index"]).astype(np.int32)
    edge_type = np.asarray(inputs["edge_type"]).astype(np.int32)
    rel_index = np.asarray(inputs["rel_index"]).astype(np.int32)
    unseen = np.asarray(inputs["unseen_index"]).astype(np.int32)

    src, dst = edge_index[0], edge_index[1]
    # combined att || rel_emb table, valid when rel_index == edge_type % R
    fused_rel = bool(np.array_equal(rel_index, edge_type % N_REL))
    comb = np.zeros((N_REL2, COMB_W), np.float32)
    comb[:, 0:N_BASES] = att
    comb[:, N_BASES:N_BASES + D_R] = rel[np.arange(N_REL2) % N_REL]

    iota = (np.arange(1, PAD + 1, dtype=np.float32)).reshape(P, F)
    ub = np.full((P, 1), unseen.reshape(()), np.int32)
    pk_w = 2 if fused_rel else 4

    in_maps = []
    for c in range(N_CORES):
        sl = slice(c * S, (c + 1) * S)
        dpad = np.full((PAD,), -1, np.int32)
        dpad[:S] = dst[sl]
        packed = np.zeros((S, pk_w), np.int32)
        packed[:, 0] = src[sl]
        packed[:, 1] = edge_type[sl]
        if not fused_rel:
            packed[:, 2] = rel_index[sl]
        m = {
            "dst": dpad.reshape(P, F),
            "iota": iota,
            "packed": packed,
            "node_id": node_id,
            "entity": ent,
            "comb": comb,
            "basis_ent0": np.ascontiguousarray(basis[0, :D_E]),
            "basis_ent1": np.ascontiguousarray(basis[1, :D_E]),
            "basis_rel0": np.ascontiguousarray(basis[0, D_E:]),
            "basis_rel1": np.ascontiguousarray(basis[1, D_E:]),
            "unseen": ub,
        }
        if not fused_rel:
            m["rel"] = rel
        in_maps.append(m)

    res = bass_utils.run_bass_kernel_spmd(
        _get_nc(fused_rel), in_maps, core_ids=list(range(N_CORES)),
    )
    LAST_RESULTS = res

    total = np.zeros(D_E, np.float32)
    cnt = 0.0
    for r in res.results:
        total = total + r["partial"][:, 0]
        cnt += float(r["cnt"][0, 0])
    out = np.maximum(total / np.float32(max(cnt, 1.0)), np.float32(0.0))
    return out.astype(np.float32)


# revision 6
# speedup vs baseline: 1.3908x; 1.2827x over previous
"""Trainium2 Bass kernel for nn_EntityEmbedding_18433999634983.

Reference semantics: RGCN-style basis-decomposed message passing with
scatter-mean aggregation, but the final output is only row `unseen_index`
of the aggregated node matrix:

    out = relu( (sum_{e: dst[e]==u} msg_e) / max(#{e: dst[e]==u}, 1) )
    msg_e = sum_b att[edge_type[e], b] * concat(x[src[e]], rel_emb[rel_index[e]]) @ basis[b]

Only edges with dst == unseen_index contribute (~20 of 1M for uniform dst).
Per core (edges sharded 8 ways):
  1. stream the dst shard (int16) through SBUF as a [128, 977] tile, compare
     against unseen_index (mask), multiply by an fp16 column-iota, and pull
     out the matching edges with a per-partition top-8 max — all on device;
  2. reconstruct global edge ids (col + 977*partition), then indirect-DMA
     gather (src, edge_type), node_id[src], entity_table rows, and combined
     att||rel_emb rows for the (<=128) matched candidate slots;
  3. contract the gathered rows with tiny PE matmuls:
         v_b[96] = sum_e valid_e * att[et_e, b] * x_cat_e
         partial[64] = sum_b basis[b].T @ v_b
     (split into entity/relation halves so everything stays
     partition-aligned), and count matches via a ones-matmul over the
     top-8 validity mask;
  4. host sums the 8 per-core partial[64] vectors and counts, divides,
     applies ReLU.

Robustness: the ROUNDS=1 fast path covers 1 match per (core, partition)
slot (observed max for the generator distribution: 1). The kernel also
emits the exact match count and the extracted-candidate count; on the
(astronomically unlikely) mismatch the host transparently reruns a
ROUNDS=8 program variant. Similarly the combined att||rel table requires
rel_index == edge_type % 200 (true by construction of the reference edge
doubling); the host verifies and falls back to a separate-gather variant.
"""

import numpy as np

# ---- problem constants (hardcoded per spec) ----
N_CORES = 8
E = 1_000_000
S = E // N_CORES          # 125_000 edges per core
P = 128
F = 977                   # ceil(S / P)
PAD = P * F               # 125_056
N_NODES = 50_000
N_ENT = 200_000
D_E = 64
D_R = 32
IN_CH = D_E + D_R         # 96
N_REL2 = 400              # 2R (att rows)
N_REL = 200               # R  (relation_embedding rows)
N_BASES = 2
COMB_W = 36               # att (2) + rel_emb (32) + pad (2)

import os
USE_16BIT = os.environ.get("K16", "0") == "1"

_CACHE = {}
LAST_RESULTS = None       # BassKernelResults of the most recent run (for test.py)


def _build_program(fused_rel: bool, rounds: int):
    import concourse.bacc as bacc
    import concourse.tile as tile
    import concourse.mybir as mybir
    from concourse.bass import IndirectOffsetOnAxis

    f32 = mybir.dt.float32
    f16 = mybir.dt.float16
    i32 = mybir.dt.int32
    i16 = mybir.dt.int16

    nc = bacc.Bacc("TRN2", target_bir_lowering=False, debug=False)

    idt = i16 if USE_16BIT else i32
    fdt = f16 if USE_16BIT else f32
    pk_w = 2 if fused_rel else 4
    dst_d = nc.dram_tensor("dst", [P, F], idt, kind="ExternalInput")
    iota_d = nc.dram_tensor("iota", [P, F], fdt, kind="ExternalInput")
    packed_d = nc.dram_tensor("packed", [S, pk_w], i32, kind="ExternalInput")
    nid_d = nc.dram_tensor("node_id", [N_NODES, 1], i32, kind="ExternalInput")
    ent_d = nc.dram_tensor("entity", [N_ENT, D_E], f32, kind="ExternalInput")
    comb_d = nc.dram_tensor("comb", [N_REL2, COMB_W], f32, kind="ExternalInput")
    rel_d = (None if fused_rel else
             nc.dram_tensor("rel", [N_REL, D_R], f32, kind="ExternalInput"))
    # basis halves packed into one tensor, all partition-0-aligned:
    # [0:64, 0:64]=basis_ent0  [0:64, 64:128]=basis_ent1
    # [0:32, 128:192]=basis_rel0  [0:32, 192:256]=basis_rel1
    par_d = nc.dram_tensor("params", [D_E, 4 * D_E], f32, kind="ExternalInput")
    uns_d = nc.dram_tensor("unseen", [P, 1], idt, kind="ExternalInput")
    pb_d = nc.dram_tensor("pbase", [P, 1], f32, kind="ExternalInput")

    part_d = nc.dram_tensor("partial", [D_E, 1], f32, kind="ExternalOutput")
    cnt_d = nc.dram_tensor("cnts", [1, 2], f32, kind="ExternalOutput")

    X = mybir.AxisListType.X
    OP = mybir.AluOpType

    with tile.TileContext(nc) as tc:
        with (
            tc.tile_pool(name="sbuf", bufs=1) as sb,
            tc.tile_pool(name="psum", bufs=1, space="PSUM") as ps,
        ):
            # ---- input DMAs: big dst on Sync, the rest on Scalar's HWDGE ----
            dst_t = sb.tile([P, F], idt)
            nc.sync.dma_start(dst_t[:], dst_d[:])
            iota_t = sb.tile([P, F], fdt)
            nc.sync.dma_start(iota_t[:], iota_d[:])
            ub = sb.tile([P, 1], idt)
            nc.sync.dma_start(ub[:], uns_d[:])
            pb = sb.tile([P, 1], f32)
            nc.sync.dma_start(pb[:], pb_d[:])
            par_t = sb.tile([D_E, 4 * D_E], f32)
            nc.sync.dma_start(par_t[:], par_d[:])

            # ---- dense phase: mask, masked-iota, top-8 extraction ----
            mask = sb.tile([P, F], fdt)
            nc.vector.tensor_tensor(
                out=mask[:], in0=dst_t[:], in1=ub[:].to_broadcast([P, F]),
                op=OP.is_equal,
            )
            mi = sb.tile([P, F], fdt)
            nc.vector.tensor_tensor(out=mi[:], in0=mask[:], in1=iota_t[:], op=OP.mult)
            top8 = sb.tile([P, 8], fdt)
            nc.vector.max(top8[:], mi[:])

            # validity of each of the top-8 candidates (1.0 / 0.0), f32
            c8 = sb.tile([P, 8], f32)
            nc.vector.tensor_scalar(
                out=c8[:], in0=top8[:], scalar1=0.5, scalar2=None, op0=OP.is_gt,
            )
            # cnt2 col 0: exact per-partition match count; col 1: extracted count
            cnt2 = sb.tile([P, 2], f32)
            nc.vector.reduce_sum(out=cnt2[:, 0:1], in_=c8[:], axis=X)
            nc.vector.reduce_sum(out=cnt2[:, 1:2], in_=c8[:, 0:rounds], axis=X)
            ones = sb.tile([P, 1], f32)
            nc.vector.memset(ones[:], 1.0)

            # ---- sparse gather rounds ----
            ents, rels, coefs = [], [], []
            for r in range(rounds):
                col = top8[:, r:r + 1]
                # local edge id = (col - 1, clamped) + 977 * partition
                idf = sb.tile([P, 1], f32, tag=f"idf{r}")
                nc.vector.tensor_scalar(
                    out=idf[:], in0=col, scalar1=-1.0, scalar2=0.0,
                    op0=OP.add, op1=OP.max,
                )
                idg = sb.tile([P, 1], f32, tag=f"idg{r}")
                nc.vector.tensor_tensor(out=idg[:], in0=idf[:], in1=pb[:], op=OP.add)
                idx = sb.tile([P, 1], i32, tag=f"idx{r}")
                nc.vector.tensor_copy(idx[:], idg[:])

                pk = sb.tile([P, pk_w], i32, tag=f"pk{r}")
                nc.gpsimd.indirect_dma_start(
                    out=pk[:], out_offset=None, in_=packed_d[:],
                    in_offset=IndirectOffsetOnAxis(ap=idx[:, :1], axis=0),
                )
                nidg = sb.tile([P, 1], i32, tag=f"nidg{r}")
                nc.gpsimd.indirect_dma_start(
                    out=nidg[:], out_offset=None, in_=nid_d[:],
                    in_offset=IndirectOffsetOnAxis(ap=pk[:, 0:1], axis=0),
                )
                entg = sb.tile([P, D_E], f32, tag=f"entg{r}")
                nc.gpsimd.indirect_dma_start(
                    out=entg[:], out_offset=None, in_=ent_d[:],
                    in_offset=IndirectOffsetOnAxis(ap=nidg[:, :1], axis=0),
                )
                combg = sb.tile([P, COMB_W], f32, tag=f"combg{r}")
                nc.gpsimd.indirect_dma_start(
                    out=combg[:], out_offset=None, in_=comb_d[:],
                    in_offset=IndirectOffsetOnAxis(ap=pk[:, 1:2], axis=0),
                )
                if fused_rel:
                    relg = combg[:, N_BASES:N_BASES + D_R]
                else:
                    relg_t = sb.tile([P, D_R], f32, tag=f"relg{r}")
                    nc.gpsimd.indirect_dma_start(
                        out=relg_t[:], out_offset=None, in_=rel_d[:],
                        in_offset=IndirectOffsetOnAxis(ap=pk[:, 2:3], axis=0),
                    )
                    relg = relg_t[:]
                cs = []
                for b in range(N_BASES):
                    c = sb.tile([P, 1], f32, tag=f"coef{r}{b}")
                    nc.vector.tensor_tensor(
                        out=c[:], in0=combg[:, b:b + 1], in1=c8[:, r:r + 1],
                        op=OP.mult,
                    )
                    cs.append(c)
                ents.append(entg)
                rels.append(relg)
                coefs.append(cs)

            # ---- contraction matmuls ----
            be = [par_t[0:D_E, 0:D_E], par_t[0:D_E, D_E:2 * D_E]]
            br = [par_t[0:D_R, 2 * D_E:3 * D_E], par_t[0:D_R, 3 * D_E:4 * D_E]]
            out_ps = ps.tile([D_E, 1], f32)
            for b in range(N_BASES):
                ve_ps = ps.tile([D_E, 1], f32, tag=f"ve{b}")
                vr_ps = ps.tile([D_R, 1], f32, tag=f"vr{b}")
                for r in range(rounds):
                    nc.tensor.matmul(
                        out=ve_ps[:], lhsT=ents[r][:], rhs=coefs[r][b][:],
                        start=(r == 0), stop=(r == rounds - 1),
                    )
                    nc.tensor.matmul(
                        out=vr_ps[:], lhsT=rels[r], rhs=coefs[r][b][:],
                        start=(r == 0), stop=(r == rounds - 1),
                    )
                ve_sb = sb.tile([D_E, 1], f32, tag=f"ves{b}")
                nc.vector.tensor_copy(ve_sb[:], ve_ps[:])
                vr_sb = sb.tile([D_R, 1], f32, tag=f"vrs{b}")
                nc.vector.tensor_copy(vr_sb[:], vr_ps[:])
                nc.tensor.matmul(
                    out=out_ps[:], lhsT=be[b], rhs=ve_sb[:],
                    start=(b == 0), stop=False,
                )
                nc.tensor.matmul(
                    out=out_ps[:], lhsT=br[b], rhs=vr_sb[:],
                    start=False, stop=(b == N_BASES - 1),
                )

            cnt_ps = ps.tile([1, 2], f32)
            nc.tensor.matmul(
                out=cnt_ps[:], lhsT=ones[:], rhs=cnt2[:], start=True, stop=True,
            )

            po = sb.tile([D_E, 1], f32)
            nc.vector.tensor_copy(po[:], out_ps[:])
            nc.sync.dma_start(part_d[:], po[:])
            co = sb.tile([1, 2], f32)
            nc.vector.tensor_copy(co[:], cnt_ps[:])
            nc.sync.dma_start(cnt_d[:], co[:])

    nc.finalize()
    return nc


def _get_nc(fused_rel: bool, rounds: int):
    key = (fused_rel, rounds)
    if key not in _CACHE:
        _CACHE[key] = _build_program(fused_rel, rounds)
    return _CACHE[key]


def _run(fused_rel, rounds, shard_args):
    from concourse import bass_utils

    (dst, src, edge_type, rel_index, node_id, ent, comb, rel, params, iota,
     ub, pb) = shard_args
    pk_w = 2 if fused_rel else 4
    in_maps = []
    for c in range(N_CORES):
        sl = slice(c * S, (c + 1) * S)
        hidt = np.int16 if USE_16BIT else np.int32
        dpad = np.full((PAD,), -1, hidt)
        dpad[:S] = dst[sl].astype(hidt)
        packed = np.zeros((S, pk_w), np.int32)
        packed[:, 0] = src[sl]
        packed[:, 1] = edge_type[sl]
        if not fused_rel:
            packed[:, 2] = rel_index[sl]
        m = {
            "dst": dpad.reshape(P, F),
            "iota": iota,
            "packed": packed,
            "node_id": node_id,
            "entity": ent,
            "comb": comb,
            "params": params,
            "unseen": ub,
            "pbase": pb,
        }
        if not fused_rel:
            m["rel"] = rel
        in_maps.append(m)

    return bass_utils.run_bass_kernel_spmd(
        _get_nc(fused_rel, rounds), in_maps, core_ids=list(range(N_CORES)),
    )


def kernel(**inputs) -> np.ndarray:
    global LAST_RESULTS

    ent = np.ascontiguousarray(np.asarray(inputs["entity_table"], np.float32))
    rel = np.ascontiguousarray(np.asarray(inputs["relation_embedding"], np.float32))
    att = np.ascontiguousarray(np.asarray(inputs["att"], np.float32))
    basis = np.asarray(inputs["basis"], np.float32)
    node_id = np.asarray(inputs["node_id"]).astype(np.int32).reshape(N_NODES, 1)
    edge_index = np.asarray(inputs["edge_index"]).astype(np.int32)
    edge_type = np.asarray(inputs["edge_type"]).astype(np.int32)
    rel_index = np.asarray(inputs["rel_index"]).astype(np.int32)
    unseen = int(np.asarray(inputs["unseen_index"]).reshape(()))

    src, dst = edge_index[0], edge_index[1]
    # combined att || rel_emb table, valid when rel_index == edge_type % R
    fused_rel = bool(np.array_equal(rel_index, edge_type % N_REL))
    comb = np.zeros((N_REL2, COMB_W), np.float32)
    comb[:, 0:N_BASES] = att
    comb[:, N_BASES:N_BASES + D_R] = rel[np.arange(N_REL2) % N_REL]

    params = np.zeros((D_E, 4 * D_E), np.float32)
    params[:, 0:D_E] = basis[0, :D_E]
    params[:, D_E:2 * D_E] = basis[1, :D_E]
    params[:D_R, 2 * D_E:3 * D_E] = basis[0, D_E:]
    params[:D_R, 3 * D_E:4 * D_E] = basis[1, D_E:]

    iota = np.tile(np.arange(1, F + 1, dtype=np.float16 if USE_16BIT else np.float32), (P, 1))
    ub = np.full((P, 1), unseen, np.int16 if USE_16BIT else np.int32)
    pb = (np.arange(P, dtype=np.float32) * F).reshape(P, 1)

    shard_args = (dst, src, edge_type, rel_index, node_id, ent, comb, rel,
                  params, iota, ub, pb)

    res = _run(fused_rel, 1, shard_args)
    cnt_all = sum(float(r["cnts"][0, 0]) for r in res.results)
    cnt_ext = sum(float(r["cnts"][0, 1]) for r in res.results)
    if cnt_all != cnt_ext:
        # >1 match landed in one (core, partition) slot: rerun with 8 rounds
        res = _run(fused_rel, 8, shard_args)
        cnt_all = sum(float(r["cnts"][0, 0]) for r in res.results)
        cnt_ext = sum(float(r["cnts"][0, 1]) for r in res.results)
        assert cnt_all == cnt_ext, (cnt_all, cnt_ext)
    LAST_RESULTS = res

    total = np.zeros(D_E, np.float32)
    for r in res.results:
        total = total + r["partial"][:, 0]
    out = np.maximum(total / np.float32(max(cnt_all, 1.0)), np.float32(0.0))
    return out.astype(np.float32)


# revision 8
# speedup vs baseline: 1.4950x; 1.0749x over previous
"""Trainium2 Bass kernel for nn_EntityEmbedding_18433999634983.

Reference semantics: RGCN-style basis-decomposed message passing with
scatter-mean aggregation, but the final output is only row `unseen_index`
of the aggregated node matrix:

    out = relu( (sum_{e: dst[e]==u} msg_e) / max(#{e: dst[e]==u}, 1) )
    msg_e = sum_b att[edge_type[e], b] * concat(x[src[e]], rel_emb[rel_index[e]]) @ basis[b]

Only edges with dst == unseen_index contribute (~20 of 1M for uniform dst).
Per core (edges sharded 8 ways):
  1. stream the dst shard (int16, packed beside an fp16 column-iota in one
     DMA) through SBUF as a [128, 977] tile, compare against unseen_index
     (mask), multiply by the iota, and pull out the matching edges with a
     per-partition top-8 max — all on device;
  2. reconstruct global edge ids (col + 977*partition), then indirect-DMA
     gather (src, edge_type), node_id[src], entity_table rows, and combined
     att||rel_emb rows for the (<=128) matched candidate slots;
  3. contract the gathered rows with tiny PE matmuls:
         v_b[96] = sum_e valid_e * att[et_e, b] * x_cat_e   (both b at once)
         partial[64] = sum_b basis[b].T @ v_b
     (split into entity/relation halves so everything stays
     partition-aligned), and count matches via a ones-matmul over the
     top-8 validity mask;
  4. host sums the 8 per-core partial[64] vectors and counts, divides,
     applies ReLU.

Robustness: the ROUNDS=1 fast path covers 1 match per (core, partition)
slot (observed max for the generator distribution: 1). The kernel also
emits the exact match count and the extracted-candidate count; on the
(astronomically unlikely) mismatch the host transparently reruns a
ROUNDS=8 program variant. Similarly the combined att||rel table requires
rel_index == edge_type % 200 (true by construction of the reference edge
doubling); the host verifies and falls back to a separate-gather variant.
"""

import numpy as np

# ---- problem constants (hardcoded per spec) ----
N_CORES = 8
E = 1_000_000
S = E // N_CORES          # 125_000 edges per core
P = 128
F = 977                   # ceil(S / P)
PAD = P * F               # 125_056
N_NODES = 50_000
N_ENT = 200_000
D_E = 64
D_R = 32
IN_CH = D_E + D_R         # 96
N_REL2 = 400              # 2R (att rows)
N_REL = 200               # R  (relation_embedding rows)
N_BASES = 2
COMB_W = 36               # att (2) + rel_emb (32) + pad (2)
PAR_W = 4 * D_E + 2       # basis quads + pbase col + unseen col

_CACHE = {}
LAST_RESULTS = None       # BassKernelResults of the most recent run (for test.py)


def _build_program(fused_rel: bool, rounds: int):
    import concourse.bacc as bacc
    import concourse.tile as tile
    import concourse.mybir as mybir
    from concourse.bass import IndirectOffsetOnAxis

    f32 = mybir.dt.float32
    f16 = mybir.dt.float16
    i32 = mybir.dt.int32
    i16 = mybir.dt.int16

    nc = bacc.Bacc("TRN2", target_bir_lowering=False, debug=False)

    pk_w = 2 if fused_rel else 4
    # cols 0:977 dst (int16), 977:1954 column-iota 1..977 (fp16 bits)
    dio_d = nc.dram_tensor("dstio", [P, 2 * F], i16, kind="ExternalInput")
    packed_d = nc.dram_tensor("packed", [S, pk_w], i32, kind="ExternalInput")
    nid_d = nc.dram_tensor("node_id", [N_NODES, 1], i32, kind="ExternalInput")
    ent_d = nc.dram_tensor("entity", [N_ENT, D_E], f32, kind="ExternalInput")
    comb_d = nc.dram_tensor("comb", [N_REL2, COMB_W], f32, kind="ExternalInput")
    rel_d = (None if fused_rel else
             nc.dram_tensor("rel", [N_REL, D_R], f32, kind="ExternalInput"))
    # [0:64, 0:64]=basis_ent0  [0:64, 64:128]=basis_ent1
    # [0:32, 128:192]=basis_rel0  [0:32, 192:256]=basis_rel1
    # [:, 256]=pbase (f32)  [:, 257]=unseen (int16 bits in low half)
    par_d = nc.dram_tensor("params", [P, PAR_W], f32, kind="ExternalInput")

    # col 0: partial[64]; col 1 rows 0:2: [cnt_exact, cnt_extracted]
    out_d = nc.dram_tensor("out", [D_E, 2], f32, kind="ExternalOutput")

    X = mybir.AxisListType.X
    OP = mybir.AluOpType

    with tile.TileContext(nc) as tc:
        with (
            tc.tile_pool(name="sbuf", bufs=1) as sb,
            tc.tile_pool(name="psum", bufs=1, space="PSUM") as ps,
        ):
            dio_t = sb.tile([P, 2 * F], i16)
            nc.sync.dma_start(dio_t[:], dio_d[:])
            par_t = sb.tile([P, PAR_W], f32)
            nc.sync.dma_start(par_t[:], par_d[:])

            dst_t = dio_t[:, 0:F]
            iota_t = dio_t[:, F:2 * F].bitcast(f16)
            pb = par_t[:, 4 * D_E:4 * D_E + 1]
            ub = par_t[:, 4 * D_E + 1:4 * D_E + 2].bitcast(i16)[:, 0:1]

            # ---- dense phase: mask, masked-iota, top-8 extraction ----
            mask = sb.tile([P, F], f16)
            nc.vector.tensor_tensor(
                out=mask[:], in0=dst_t, in1=ub.to_broadcast([P, F]),
                op=OP.is_equal,
            )
            mi = sb.tile([P, F], f16)
            nc.vector.tensor_tensor(out=mi[:], in0=mask[:], in1=iota_t, op=OP.mult)
            top8 = sb.tile([P, 8], f16)
            nc.vector.max(top8[:], mi[:])

            # validity of each of the top-8 candidates (1.0 / 0.0), f32
            c8 = sb.tile([P, 8], f32)
            nc.vector.tensor_scalar(
                out=c8[:], in0=top8[:], scalar1=0.5, scalar2=None, op0=OP.is_gt,
            )
            # cnt2 col 0: exact per-partition match count; col 1: extracted count
            cnt2 = sb.tile([P, 2], f32)
            nc.vector.reduce_sum(out=cnt2[:, 0:1], in_=c8[:], axis=X)
            nc.vector.reduce_sum(out=cnt2[:, 1:2], in_=c8[:, 0:rounds], axis=X)
            ones = sb.tile([P, 1], f32)
            nc.vector.memset(ones[:], 1.0)

            # ---- sparse gather rounds ----
            ents, rels, coefs = [], [], []
            for r in range(rounds):
                col = top8[:, r:r + 1]
                # local edge id = (col - 1, clamped) + 977 * partition
                idf = sb.tile([P, 1], f32, tag=f"idf{r}")
                nc.vector.tensor_scalar(
                    out=idf[:], in0=col, scalar1=-1.0, scalar2=0.0,
                    op0=OP.add, op1=OP.max,
                )
                idg = sb.tile([P, 1], f32, tag=f"idg{r}")
                nc.vector.tensor_tensor(out=idg[:], in0=idf[:], in1=pb, op=OP.add)
                idx = sb.tile([P, 1], i32, tag=f"idx{r}")
                nc.vector.tensor_copy(idx[:], idg[:])

                pk = sb.tile([P, pk_w], i32, tag=f"pk{r}")
                nc.gpsimd.indirect_dma_start(
                    out=pk[:], out_offset=None, in_=packed_d[:],
                    in_offset=IndirectOffsetOnAxis(ap=idx[:, :1], axis=0),
                )
                nidg = sb.tile([P, 1], i32, tag=f"nidg{r}")
                nc.gpsimd.indirect_dma_start(
                    out=nidg[:], out_offset=None, in_=nid_d[:],
                    in_offset=IndirectOffsetOnAxis(ap=pk[:, 0:1], axis=0),
                )
                entg = sb.tile([P, D_E], f32, tag=f"entg{r}")
                nc.gpsimd.indirect_dma_start(
                    out=entg[:], out_offset=None, in_=ent_d[:],
                    in_offset=IndirectOffsetOnAxis(ap=nidg[:, :1], axis=0),
                )
                combg = sb.tile([P, COMB_W], f32, tag=f"combg{r}")
                nc.gpsimd.indirect_dma_start(
                    out=combg[:], out_offset=None, in_=comb_d[:],
                    in_offset=IndirectOffsetOnAxis(ap=pk[:, 1:2], axis=0),
                )
                if fused_rel:
                    relg = combg[:, N_BASES:N_BASES + D_R]
                else:
                    relg_t = sb.tile([P, D_R], f32, tag=f"relg{r}")
                    nc.gpsimd.indirect_dma_start(
                        out=relg_t[:], out_offset=None, in_=rel_d[:],
                        in_offset=IndirectOffsetOnAxis(ap=pk[:, 2:3], axis=0),
                    )
                    relg = relg_t[:]
                # both bases' coefficients at once: [att0*valid, att1*valid]
                c2 = sb.tile([P, N_BASES], f32, tag=f"c2{r}")
                nc.vector.tensor_tensor(
                    out=c2[:], in0=combg[:, 0:N_BASES],
                    in1=c8[:, r:r + 1].to_broadcast([P, N_BASES]), op=OP.mult,
                )
                ents.append(entg)
                rels.append(relg)
                coefs.append(c2)

            # ---- contraction matmuls ----
            be = [par_t[0:D_E, 0:D_E], par_t[0:D_E, D_E:2 * D_E]]
            br = [par_t[0:D_R, 2 * D_E:3 * D_E], par_t[0:D_R, 3 * D_E:4 * D_E]]
            ve_ps = ps.tile([D_E, N_BASES], f32)
            vr_ps = ps.tile([D_R, N_BASES], f32)
            for r in range(rounds):
                nc.tensor.matmul(
                    out=ve_ps[:], lhsT=ents[r][:], rhs=coefs[r][:],
                    start=(r == 0), stop=(r == rounds - 1),
                )
                nc.tensor.matmul(
                    out=vr_ps[:], lhsT=rels[r], rhs=coefs[r][:],
                    start=(r == 0), stop=(r == rounds - 1),
                )
            ve_sb = sb.tile([D_E, N_BASES], f32)
            nc.vector.tensor_copy(ve_sb[:], ve_ps[:])
            vr_sb = sb.tile([D_R, N_BASES], f32)
            nc.vector.tensor_copy(vr_sb[:], vr_ps[:])

            out_ps = ps.tile([D_E, 1], f32)
            for b in range(N_BASES):
                nc.tensor.matmul(
                    out=out_ps[:], lhsT=be[b], rhs=ve_sb[:, b:b + 1],
                    start=(b == 0), stop=False,
                )
                nc.tensor.matmul(
                    out=out_ps[:], lhsT=br[b], rhs=vr_sb[:, b:b + 1],
                    start=False, stop=(b == N_BASES - 1),
                )
            cnt_ps = ps.tile([2, 1], f32)
            nc.tensor.matmul(
                out=cnt_ps[:], lhsT=cnt2[:], rhs=ones[:], start=True, stop=True,
            )

            po = sb.tile([D_E, 2], f32)
            nc.vector.tensor_copy(po[:, 0:1], out_ps[:])
            nc.vector.tensor_copy(po[0:2, 1:2], cnt_ps[:])
            nc.sync.dma_start(out_d[:], po[:])

    nc.finalize()
    return nc


def _get_nc(fused_rel: bool, rounds: int):
    key = (fused_rel, rounds)
    if key not in _CACHE:
        _CACHE[key] = _build_program(fused_rel, rounds)
    return _CACHE[key]


def _run(fused_rel, rounds, shard_args):
    from concourse import bass_utils

    (dst, src, edge_type, rel_index, node_id, ent, comb, rel, params) = shard_args
    pk_w = 2 if fused_rel else 4
    iota16 = np.arange(1, F + 1, dtype=np.float16).view(np.int16)
    in_maps = []
    for c in range(N_CORES):
        sl = slice(c * S, (c + 1) * S)
        dio = np.empty((P, 2 * F), np.int16)
        dpad = np.full((PAD,), -1, np.int16)
        dpad[:S] = dst[sl].astype(np.int16)
        dio[:, 0:F] = dpad.reshape(P, F)
        dio[:, F:2 * F] = iota16
        packed = np.zeros((S, pk_w), np.int32)
        packed[:, 0] = src[sl]
        packed[:, 1] = edge_type[sl]
        if not fused_rel:
            packed[:, 2] = rel_index[sl]
        m = {
            "dstio": dio,
            "packed": packed,
            "node_id": node_id,
            "entity": ent,
            "comb": comb,
            "params": params,
        }
        if not fused_rel:
            m["rel"] = rel
        in_maps.append(m)

    return bass_utils.run_bass_kernel_spmd(
        _get_nc(fused_rel, rounds), in_maps, core_ids=list(range(N_CORES)),
    )


def _prep_params(basis, unseen):
    params = np.zeros((P, PAR_W), np.float32)
    params[:D_E, 0:D_E] = basis[0, :D_E]
    params[:D_E, D_E:2 * D_E] = basis[1, :D_E]
    params[:D_R, 2 * D_E:3 * D_E] = basis[0, D_E:]
    params[:D_R, 3 * D_E:4 * D_E] = basis[1, D_E:]
    params[:, 4 * D_E] = np.arange(P, dtype=np.float32) * F
    ucol = np.zeros((P, 2), np.int16)
    ucol[:, 0] = unseen
    params[:, 4 * D_E + 1] = ucol.view(np.float32)[:, 0]
    return params


def kernel(**inputs) -> np.ndarray:
    global LAST_RESULTS

    ent = np.ascontiguousarray(np.asarray(inputs["entity_table"], np.float32))
    rel = np.ascontiguousarray(np.asarray(inputs["relation_embedding"], np.float32))
    att = np.ascontiguousarray(np.asarray(inputs["att"], np.float32))
    basis = np.asarray(inputs["basis"], np.float32)
    node_id = np.asarray(inputs["node_id"]).astype(np.int32).reshape(N_NODES, 1)
    edge_index = np.asarray(inputs["edge_index"]).astype(np.int32)
    edge_type = np.asarray(inputs["edge_type"]).astype(np.int32)
    rel_index = np.asarray(inputs["rel_index"]).astype(np.int32)
    unseen = int(np.asarray(inputs["unseen_index"]).reshape(()))

    src, dst = edge_index[0], edge_index[1]
    # combined att || rel_emb table, valid when rel_index == edge_type % R
    fused_rel = bool(np.array_equal(rel_index, edge_type % N_REL))
    comb = np.zeros((N_REL2, COMB_W), np.float32)
    comb[:, 0:N_BASES] = att
    comb[:, N_BASES:N_BASES + D_R] = rel[np.arange(N_REL2) % N_REL]
    params = _prep_params(basis, unseen)

    shard_args = (dst, src, edge_type, rel_index, node_id, ent, comb, rel, params)

    res = _run(fused_rel, 1, shard_args)
    cnt_all = sum(float(r["out"][0, 1]) for r in res.results)
    cnt_ext = sum(float(r["out"][1, 1]) for r in res.results)
    if cnt_all != cnt_ext:
        # >1 match landed in one (core, partition) slot: rerun with 8 rounds
        res = _run(fused_rel, 8, shard_args)
        cnt_all = sum(float(r["out"][0, 1]) for r in res.results)
        cnt_ext = sum(float(r["out"][1, 1]) for r in res.results)
        assert cnt_all == cnt_ext, (cnt_all, cnt_ext)
    LAST_RESULTS = res

    total = np.zeros(D_E, np.float32)
    for r in res.results:
        total = total + r["out"][:, 0]
    out = np.maximum(total / np.float32(max(cnt_all, 1.0)), np.float32(0.0))
    return out.astype(np.float32)
